# revision 1
# baseline (speedup 1.0000x reference)
"""KPConv Bass/Trainium2 kernel.

out[m,d] = sum_k ( sum_h infl[m,h,k] * s_feats[idx[m,h],:] ) @ W[k]
infl[m,h,k] = relu(1 - |s_pts[idx[m,h]] - q_pts[m] - kp[k]| / SIGMA)

Sharding: query points M=50000 split 8 ways (6250/core, padded to 6272 =
49 blocks x 128 points). s_pts / s_feats / weights / kernel_points
replicated per core.

Per-core dataflow, per block of 128 query points (= 32 "tiles" of 4
points x 32 neighbors = 128 edges each):
  1. one indirect-DMA gather of all 4096 neighbor feature rows (512B each)
     into SBUF [128, 4096] (tile t at cols t*128) - edge e of tile t sits
     in partition e.
  2. one indirect-DMA gather of neighbor coords [128, 96].
  3. influence on DVE/ACT: delta, (delta-kp)^2, segmented reduce, sqrt,
     relu affine -> infl [128, 32*15].
  4. step A on PE: per tile t, matmul(lhsT=nf_t [128e,128c],
     rhs=block-diag influence [128e, 60]) -> PSUM wfT [128c, m*15+k].
  5. step B on PE: per k, matmul(lhsT=wfT[:, k::15] [c,m], rhs=W[k] [c,d])
     accumulating over k -> PSUM [128m, 128d] -> SBUF -> DRAM.
"""

import sys

sys.path.insert(0, "/opt/trn_rl_repo")

import numpy as np

# ---------------------------------------------------------------- constants
N_CORES = 8
M_TOTAL = 50000
N_SUP = 50000
H = 32
C = 128
K = 15
SIGMA = 2.0

M_CORE = M_TOTAL // N_CORES          # 6250
P = 128                              # partitions / points per block
NB = (M_CORE + P - 1) // P           # 49 blocks
M_PAD = NB * P                       # 6272
G = 4                                # points per step-A matmul tile
NT = P // G                          # 32 tiles per block

_compiled = None


def _build_bass(nb=NB, n_sup=N_SUP, compile=True, repeats=1, parts="all"):
    """Build + compile the per-core SPMD Bass program."""
    from contextlib import ExitStack

    import concourse.bacc as bacc
    import concourse.mybir as mybir
    import concourse.tile as tile
    from concourse import bass

    f32 = mybir.dt.float32
    i32 = mybir.dt.int32
    NB = nb
    N_SUP_ = n_sup

    nc = bacc.Bacc(
        "TRN2",
        target_bir_lowering=False,
        debug=False,
        enable_asserts=False,
        num_devices=N_CORES,
    )

    q_blk_d = nc.dram_tensor("q_blk", (NB, P, NT * 3), f32, kind="ExternalInput")
    inds_d = nc.dram_tensor("inds_blk", (NB, P, NT), i32, kind="ExternalInput")
    sfeats_d = nc.dram_tensor("s_feats", (N_SUP_, C), f32, kind="ExternalInput")
    spts_d = nc.dram_tensor("s_pts", (N_SUP_, 3), f32, kind="ExternalInput")
    w_d = nc.dram_tensor("weights", (K, C, C), f32, kind="ExternalInput")
    kp_d = nc.dram_tensor("kp_rep", (P, K * 3), f32, kind="ExternalInput")
    mask_d = nc.dram_tensor("mask60", (P, G * K), f32, kind="ExternalInput")
    out_d = nc.dram_tensor("out", (NB, P, C), f32, kind="ExternalOutput")

    sub = mybir.AluOpType.subtract
    mult = mybir.AluOpType.mult

    with tile.TileContext(nc) as tc, ExitStack() as ctx:
        const = ctx.enter_context(tc.tile_pool(name="const", bufs=1))
        io = ctx.enter_context(tc.tile_pool(name="io", bufs=2))
        mid = ctx.enter_context(tc.tile_pool(name="mid", bufs=2))
        psa = ctx.enter_context(tc.tile_pool(name="psa", bufs=1, space="PSUM"))
        psb = ctx.enter_context(tc.tile_pool(name="psb", bufs=2, space="PSUM"))

        # constants: weights as [c, k, d], kernel points, block-diag mask
        w_sb = const.tile([P, K, C], f32)
        nc.sync.dma_start(w_sb[:], w_d.ap().rearrange("k c d -> c k d"))
        kp_sb = const.tile([P, K * 3], f32)
        nc.sync.dma_start(kp_sb[:], kp_d.ap())
        mask_sb = const.tile([P, G * K], f32)
        nc.sync.dma_start(mask_sb[:], mask_d.ap())

        do_gather = parts in ("all", "gather")
        do_compute = parts in ("all", "compute")
        for B in [b for _ in range(repeats) for b in range(NB)]:
            inds = io.tile([P, NT], i32, tag="inds")
            nc.sync.dma_start(inds[:], inds_d.ap()[B])
            qb = io.tile([P, NT * 3], f32, tag="qb")
            nc.sync.dma_start(qb[:], q_blk_d.ap()[B])

            # gather neighbor features: tile t -> cols [t*128, (t+1)*128)
            # (HW indirect DMA consumes exactly one index per partition)
            nf = io.tile([P, NT * C], f32, tag="nf")
            sg = io.tile([P, NT * 3], f32, tag="sg")
            if do_gather:
                for t in range(NT):
                    nc.gpsimd.indirect_dma_start(
                        out=nf[:, t * C : (t + 1) * C],
                        out_offset=None,
                        in_=sfeats_d.ap(),
                        in_offset=bass.IndirectOffsetOnAxis(
                            ap=inds[:, t : t + 1], axis=0
                        ),
                    )
                for t in range(NT):
                    nc.gpsimd.indirect_dma_start(
                        out=sg[:, t * 3 : (t + 1) * 3],
                        out_offset=None,
                        in_=spts_d.ap(),
                        in_offset=bass.IndirectOffsetOnAxis(
                            ap=inds[:, t : t + 1], axis=0
                        ),
                    )
            else:
                nc.gpsimd.memset(nf[:], 0.5)
                nc.gpsimd.memset(sg[:], 0.5)
            if not do_compute:
                osb0 = mid.tile([P, C], f32, tag="osb")
                nc.vector.tensor_copy(osb0[:], nf[:, :C])
                nc.sync.dma_start(out_d.ap()[B], osb0[:])
                continue

            # influence
            delta = mid.tile([P, NT * 3], f32, tag="delta")
            nc.vector.tensor_tensor(delta[:], sg[:], qb[:], op=sub)

            diff = mid.tile([P, NT * K * 3], f32, tag="diff")
            nc.vector.tensor_tensor(
                diff[:].rearrange("p (t k j) -> p t k j", k=K, j=3),
                delta[:].rearrange("p (t j) -> p t j", j=3)
                .unsqueeze(2)
                .broadcast_to([P, NT, K, 3]),
                kp_sb[:].rearrange("p (k j) -> p k j", j=3)
                .unsqueeze(1)
                .broadcast_to([P, NT, K, 3]),
                op=sub,
            )
            sq = mid.tile([P, NT * K * 3], f32, tag="sq")
            nc.vector.tensor_tensor(sq[:], diff[:], diff[:], op=mult)
            d2 = mid.tile([P, NT * K], f32, tag="d2")
            nc.vector.reduce_sum(
                out=d2[:],
                in_=sq[:].rearrange("p (tk j) -> p tk j", j=3),
                axis=mybir.AxisListType.X,
            )
            dd = mid.tile([P, NT * K], f32, tag="dd")
            nc.scalar.sqrt(dd[:], d2[:])
            infl = mid.tile([P, NT * K], f32, tag="infl")
            nc.scalar.activation(
                infl[:],
                dd[:],
                mybir.ActivationFunctionType.Relu,
                bias=1.0,
                scale=-1.0 / SIGMA,
            )

            # block-diagonal influence [p, t*60 + g*15 + k]
            bd = mid.tile([P, NT * G * K], f32, tag="bd")
            nc.vector.tensor_tensor(
                bd[:].rearrange("p (t g k) -> p t g k", g=G, k=K),
                infl[:].rearrange("p (t k) -> p t k", k=K)
                .unsqueeze(2)
                .broadcast_to([P, NT, G, K]),
                mask_sb[:].rearrange("p (g k) -> p g k", k=K)
                .unsqueeze(1)
                .broadcast_to([P, NT, G, K]),
                op=mult,
            )

            # step A: 32 matmuls -> wfT[c, m*15+k] in 4 PSUM banks
            pa = [
                psa.tile([P, 8 * G * K], f32, tag=f"psA{q}", name=f"psA{q}")
                for q in range(4)
            ]
            for t in range(NT):
                nc.tensor.matmul(
                    pa[t // 8][:, (t % 8) * (G * K) : (t % 8 + 1) * (G * K)],
                    lhsT=nf[:, t * C : (t + 1) * C],
                    rhs=bd[:, t * (G * K) : (t + 1) * (G * K)],
                    start=True,
                    stop=True,
                )
            wfT = mid.tile([P, P * K], f32, tag="wfT")
            for q in range(4):
                nc.scalar.copy(wfT[:, q * 480 : (q + 1) * 480], pa[q][:])

            # step B: accumulate over k
            outp = psb.tile([P, C], f32, tag="outp")
            wview = wfT[:].rearrange("p (m k) -> p k m", k=K)
            for k in range(K):
                nc.tensor.matmul(
                    outp[:],
                    lhsT=wview[:, k, :],
                    rhs=w_sb[:, k, :],
                    start=(k == 0),
                    stop=(k == K - 1),
                )
            osb = mid.tile([P, C], f32, tag="osb")
            nc.scalar.copy(osb[:], outp[:])
            nc.sync.dma_start(out_d.ap()[B], osb[:])

    if compile:
        nc.compile()
    return nc


def _host_prep(q_pts, s_pts, s_feats, neighb_inds, weights, kernel_points):
    """Shard + lay out inputs for the 8 cores."""
    q_pts = np.asarray(q_pts, np.float32)
    s_pts = np.asarray(s_pts, np.float32)
    s_feats = np.asarray(s_feats, np.float32)
    neighb_inds = np.asarray(neighb_inds, np.int32)
    weights = np.asarray(weights, np.float32)
    kernel_points = np.asarray(kernel_points, np.float32)

    kp_rep = np.broadcast_to(
        kernel_points.reshape(1, K * 3), (P, K * 3)
    ).copy()
    mask60 = (
        (np.arange(G * K)[None, :] // K) == (np.arange(P)[:, None] // H)
    ).astype(np.float32)

    in_maps = []
    for i in range(N_CORES):
        sl = slice(i * M_CORE, (i + 1) * M_CORE)
        q = np.zeros((M_PAD, 3), np.float32)
        q[:M_CORE] = q_pts[sl]
        idx = np.zeros((M_PAD, H), np.int32)
        idx[:M_CORE] = neighb_inds[sl]

        # inds_blk[B, g*32+h, t] = idx[B*128 + 4t + g, h]
        a = idx.reshape(NB, NT, G, H)            # [B, t, g, h]
        inds_blk = np.ascontiguousarray(
            a.transpose(0, 2, 3, 1)              # [B, g, h, t]
        ).reshape(NB, P, NT)

        # q_blk[B, g*32+h, 3t+j] = q[B*128 + 4t + g, j]
        b = q.reshape(NB, NT, G, 3)              # [B, t, g, j]
        b = b.transpose(0, 2, 1, 3)              # [B, g, t, j]
        q_blk = np.repeat(
            b.reshape(NB, G, 1, NT * 3), H, axis=2
        ).reshape(NB, P, NT * 3)

        in_maps.append(
            {
                "q_blk": np.ascontiguousarray(q_blk),
                "inds_blk": inds_blk,
                "s_feats": s_feats,
                "s_pts": s_pts,
                "weights": weights,
                "kp_rep": kp_rep,
                "mask60": mask60,
            }
        )
    return in_maps


def kernel(q_pts, s_pts, s_feats, neighb_inds, weights, kernel_points):
    global _compiled
    if _compiled is None:
        _compiled = _build_bass()
    nc = _compiled

    from concourse.bass_utils import run_bass_kernel_spmd

    in_maps = _host_prep(
        q_pts, s_pts, s_feats, neighb_inds, weights, kernel_points
    )
    res = run_bass_kernel_spmd(nc, in_maps, core_ids=list(range(N_CORES)))
    out = np.concatenate(
        [r["out"].reshape(M_PAD, C)[:M_CORE] for r in res.results], axis=0
    )
    return out.astype(np.float32)


if __name__ == "__main__":
    rng = np.random.default_rng(0)
    ins = {
        "q_pts": rng.standard_normal((M_TOTAL, 3), np.float32),
        "s_pts": rng.standard_normal((N_SUP, 3), np.float32),
        "s_feats": rng.standard_normal((N_SUP, C), np.float32),
        "neighb_inds": rng.integers(0, N_SUP, (M_TOTAL, H)).astype(np.int32),
        "weights": rng.standard_normal((K, C, C), np.float32) * 0.05,
        "kernel_points": rng.standard_normal((K, 3), np.float32),
    }
    out = kernel(**ins)
    print(out.shape, out.dtype)



# revision 6
# speedup vs baseline: 2.7959x; 2.7959x over previous
"""KPConv Bass/Trainium2 kernel.

out[m,d] = sum_k ( sum_h infl[m,h,k] * s_feats[idx[m,h],:] ) @ W[k]
infl[m,h,k] = relu(1 - |s_pts[idx[m,h]] - q_pts[m] - kp[k]| / SIGMA)

Sharding: query points M=50000 split 8 ways (6250/core, padded to 6272 =
49 blocks x 128 points). Support features / weights replicated per core.

Per-core dataflow, per block of 128 query points (= 32 tiles of 4 points
x 32 neighbors = 128 edges each; edge j of the block sits at gather slot
partition j%128, column j//128, with j = t*128 + g*32 + h):

  1. neighbor features are fetched with ONE dma_gather per block from a
     PAIR-PACKED bf16 table: tab2[p] = [feats[2p] | feats[2p+1]] (512B
     rows, 25001 rows).  Halving the row count fits the whole table in
     dma_gather's int16 index window (25000 < 32768), so a single call
     covers all slots (no split windows / placeholder rows).  dma_gather
     amortizes SWDGE descriptor generation (994ns + 0.34ns/desc per call
     vs ~1us per 128-row indirect DMA in the old kernel), and
     single_packet=False keeps each descriptor its own packet (a
     coalesced packet caps at 64 descriptors -> device error).
  2. the even/odd half-row selection rides the step-A matmul for free:
     bd is built twice, bd_e = infl*mask_e and bd_o = infl*mask_o, where
     the host-shipped per-block masks fold the block-diagonal structure
     AND the row parity: mask_e[p,(t,g)] = (g==p//32)*(idx even).
  3. influence in fp32 on DVE/ACT from host-prepped per-edge delta =
     s_pts[idx]-q (coords ride from the host, 12B/edge, vs 32 tiny 12B
     indirect DMAs per block): (delta-kp)^2, segmented reduce, sqrt,
     relu affine -> infl (bf16).
  4. step A on PE (bf16): per tile t, two accumulating matmuls
     (lhsT=even/odd half of nfp_t [128e,128c], rhs=bd_e/bd_o_t
     [128e, 60]) -> PSUM wfT [128c, m*15+k] fp32 in 4 banks.
  5. step B on PE (bf16): per k, matmul(lhsT=wfT[:, k::15] [c,m],
     rhs=W[k] [c,d]) accumulating over k -> PSUM [128m, 128d] -> SBUF
     -> DRAM.
"""

import sys

sys.path.insert(0, "/opt/trn_rl_repo")

import numpy as np

# ---------------------------------------------------------------- constants
N_CORES = 8
M_TOTAL = 50000
N_SUP = 50000
H = 32
C = 128
K = 15
SIGMA = 2.0

M_CORE = M_TOTAL // N_CORES          # 6250
P = 128                              # partitions / points per block
NB = (M_CORE + P - 1) // P           # 49 blocks
M_PAD = NB * P                       # 6272
G = 4                                # points per step-A matmul tile
NT = P // G                          # 32 tiles per block
NI = P * NT                          # 4096 gathered rows per block

NPAIR = N_SUP // 2                   # 25000 packed feature-pair rows
ZROW = NPAIR                         # all-zero pair row for padded slots

_compiled = None


def _build_bass(nb=NB, compile=True, repeats=1, parts="all"):
    """Build + compile the per-core SPMD Bass program."""
    from contextlib import ExitStack

    import concourse.bacc as bacc
    import concourse.mybir as mybir
    import concourse.tile as tile

    f32 = mybir.dt.float32
    bf16 = mybir.dt.bfloat16
    i16 = mybir.dt.int16
    NB = nb

    nc = bacc.Bacc(
        "TRN2",
        target_bir_lowering=False,
        debug=False,
        enable_asserts=False,
        num_devices=N_CORES,
    )

    tab_d = nc.dram_tensor("tab", (NPAIR + 1, 2 * C), bf16, kind="ExternalInput")
    idx_d = nc.dram_tensor("idx16", (NB, P, NI // 16), i16, kind="ExternalInput")
    delta_d = nc.dram_tensor("delta", (NB, P, NT * 3), f32, kind="ExternalInput")
    me_d = nc.dram_tensor("mask_e", (NB, P, NT * G), bf16, kind="ExternalInput")
    mo_d = nc.dram_tensor("mask_o", (NB, P, NT * G), bf16, kind="ExternalInput")
    w_d = nc.dram_tensor("weights", (K, C, C), bf16, kind="ExternalInput")
    kp_d = nc.dram_tensor("kp_rep", (P, K * 3), f32, kind="ExternalInput")
    out_d = nc.dram_tensor("out", (NB, P, C), f32, kind="ExternalOutput")

    sub = mybir.AluOpType.subtract
    mult = mybir.AluOpType.mult

    with tile.TileContext(nc) as tc, ExitStack() as ctx:
        const = ctx.enter_context(tc.tile_pool(name="const", bufs=1))
        io = ctx.enter_context(tc.tile_pool(name="io", bufs=2))
        mid = ctx.enter_context(tc.tile_pool(name="mid", bufs=2))
        psa = ctx.enter_context(tc.tile_pool(name="psa", bufs=1, space="PSUM"))
        psb = ctx.enter_context(tc.tile_pool(name="psb", bufs=2, space="PSUM"))

        # constants: weights as [c, k, d], kernel points
        w_sb = const.tile([P, K, C], bf16)
        nc.sync.dma_start(w_sb[:], w_d.ap().rearrange("k c d -> c k d"))
        kp_sb = const.tile([P, K * 3], f32)
        nc.sync.dma_start(kp_sb[:], kp_d.ap())

        do_gather = parts in ("all", "gather")
        do_compute = parts in ("all", "compute")
        for B in [b for _ in range(repeats) for b in range(NB)]:
            idx = io.tile([P, NI // 16], i16, tag="idx")
            nc.sync.dma_start(idx[:], idx_d.ap()[B])
            dlt = io.tile([P, NT * 3], f32, tag="dlt")
            nc.sync.dma_start(dlt[:], delta_d.ap()[B])
            me = io.tile([P, NT * G], bf16, tag="me")
            nc.sync.dma_start(me[:], me_d.ap()[B])
            mo = io.tile([P, NT * G], bf16, tag="mo")
            nc.sync.dma_start(mo[:], mo_d.ap()[B])

            # gather packed feature pairs: slot j -> [part j%128, col j//128]
            nfp = io.tile([P, NT, 2 * C], bf16, tag="nfp")
            if do_gather:
                nc.gpsimd.dma_gather(
                    out_ap=nfp[:],
                    in_ap=tab_d.ap(),
                    idxs_ap=idx[:],
                    num_idxs=NI,
                    num_idxs_reg=NI,
                    elem_size=2 * C,
                    single_packet=False,
                )
            else:
                nc.gpsimd.memset(nfp[:], 0.25)
            if not do_compute:
                osb0 = mid.tile([P, C], f32, tag="osb")
                nc.vector.tensor_copy(osb0[:], nfp[:, 0, :C])
                nc.sync.dma_start(out_d.ap()[B], osb0[:])
                continue

            # influence (fp32 math, bf16 out of the relu)
            diff = mid.tile([P, NT * K * 3], f32, tag="diff")
            nc.vector.tensor_tensor(
                diff[:].rearrange("p (t k j) -> p t k j", k=K, j=3),
                dlt[:].rearrange("p (t j) -> p t j", j=3)
                .unsqueeze(2)
                .broadcast_to([P, NT, K, 3]),
                kp_sb[:].rearrange("p (k j) -> p k j", j=3)
                .unsqueeze(1)
                .broadcast_to([P, NT, K, 3]),
                op=sub,
            )
            sq = mid.tile([P, NT * K * 3], f32, tag="sq")
            nc.vector.tensor_tensor(sq[:], diff[:], diff[:], op=mult)
            d2 = mid.tile([P, NT * K], f32, tag="d2")
            nc.vector.reduce_sum(
                out=d2[:],
                in_=sq[:].rearrange("p (tk j) -> p tk j", j=3),
                axis=mybir.AxisListType.X,
            )
            dd = mid.tile([P, NT * K], f32, tag="dd")
            nc.scalar.sqrt(dd[:], d2[:])
            infl = mid.tile([P, NT * K], bf16, tag="infl")
            nc.scalar.activation(
                infl[:],
                dd[:],
                mybir.ActivationFunctionType.Relu,
                bias=1.0,
                scale=-1.0 / SIGMA,
            )

            # per-parity block-diagonal influence [p, t*60 + g*15 + k]
            bde = mid.tile([P, NT * G * K], bf16, tag="bde")
            bdo = mid.tile([P, NT * G * K], bf16, tag="bdo")
            for bd, mk in ((bde, me), (bdo, mo)):
                nc.vector.tensor_tensor(
                    bd[:].rearrange("p (t g k) -> p t g k", g=G, k=K),
                    infl[:].rearrange("p (t k) -> p t k", k=K)
                    .unsqueeze(2)
                    .broadcast_to([P, NT, G, K]),
                    mk[:].rearrange("p (t g) -> p t g", g=G)
                    .unsqueeze(3)
                    .broadcast_to([P, NT, G, K]),
                    op=mult,
                )

            # step A: per tile, two accumulating matmuls (even/odd halves
            # of the gathered pair rows; the parity masks zero the wrong
            # half's contribution, PSUM accumulation merges them)
            pa = [
                psa.tile([P, 8 * G * K], f32, tag=f"psA{q}", name=f"psA{q}")
                for q in range(4)
            ]
            for t in range(NT):
                dst = pa[t // 8][:, (t % 8) * (G * K) : (t % 8 + 1) * (G * K)]
                nc.tensor.matmul(
                    dst,
                    lhsT=nfp[:, t, :C],
                    rhs=bde[:, t * (G * K) : (t + 1) * (G * K)],
                    start=True,
                    stop=False,
                )
                nc.tensor.matmul(
                    dst,
                    lhsT=nfp[:, t, C:],
                    rhs=bdo[:, t * (G * K) : (t + 1) * (G * K)],
                    start=False,
                    stop=True,
                )
            wfT = mid.tile([P, P * K], bf16, tag="wfT")
            for q in range(4):
                nc.scalar.copy(wfT[:, q * 480 : (q + 1) * 480], pa[q][:])

            # step B: accumulate over k
            outp = psb.tile([P, C], f32, tag="outp")
            wview = wfT[:].rearrange("p (m k) -> p k m", k=K)
            for k in range(K):
                nc.tensor.matmul(
                    outp[:],
                    lhsT=wview[:, k, :],
                    rhs=w_sb[:, k, :],
                    start=(k == 0),
                    stop=(k == K - 1),
                )
            osb = mid.tile([P, C], f32, tag="osb")
            nc.scalar.copy(osb[:], outp[:])
            nc.sync.dma_start(out_d.ap()[B], osb[:])

    if compile:
        nc.compile()
    return nc


def _host_prep(q_pts, s_pts, s_feats, neighb_inds, weights, kernel_points):
    """Shard + lay out inputs for the 8 cores."""
    import ml_dtypes

    bf16 = ml_dtypes.bfloat16

    q_pts = np.asarray(q_pts, np.float32)
    s_pts = np.asarray(s_pts, np.float32)
    s_feats = np.asarray(s_feats, np.float32)
    neighb_inds = np.asarray(neighb_inds, np.int64)
    weights = np.asarray(weights, np.float32)
    kernel_points = np.asarray(kernel_points, np.float32)

    # shared: pair-packed bf16 feature table + all-zero row at the end
    fb = s_feats.astype(bf16)
    tab = np.zeros((NPAIR + 1, 2 * C), bf16)
    tab[:NPAIR, :C] = fb[0::2]
    tab[:NPAIR, C:] = fb[1::2]

    kp_rep = np.broadcast_to(
        kernel_points.reshape(1, K * 3), (P, K * 3)
    ).copy()
    # g_eq[p, g] = 1 iff partition p belongs to point-group g
    g_eq = (
        (np.arange(G)[None, :] == (np.arange(P)[:, None] // H))
    ).astype(np.float32)
    w_bf = weights.astype(bf16)

    in_maps = []
    for i in range(N_CORES):
        sl = slice(i * M_CORE, (i + 1) * M_CORE)
        q = np.zeros((M_PAD, 3), np.float32)
        q[:M_CORE] = q_pts[sl]
        idx = np.full((M_PAD, H), -1, np.int64)
        idx[:M_CORE] = neighb_inds[sl]

        # gather slot j of block B (= t*128 + g*32 + h, point m =
        # B*128 + t*4 + g) -> ga[B, t, g, h]
        ga = idx.reshape(NB, NT, G, H)
        valid = ga >= 0
        i16 = np.where(valid, ga >> 1, ZROW)

        def wrap(a):  # [NB, 4096] j-order -> [NB, 128, 256] int16
            w = a.reshape(NB, NI // 16, 16).transpose(0, 2, 1)
            return np.ascontiguousarray(
                np.tile(w, (1, 8, 1)), dtype=np.int16
            )

        # parity [B, t, g, h] -> [B, p=(g,h), t]
        pe = (valid & (ga % 2 == 0)).transpose(0, 2, 3, 1)  # [B,G,H,T]
        po = (valid & (ga % 2 == 1)).transpose(0, 2, 3, 1)
        pe = pe.reshape(NB, P, NT).astype(np.float32)
        po = po.reshape(NB, P, NT).astype(np.float32)
        # mask[B, p, (t, g)] = parity[B, p, t] * g_eq[p, g]
        me = (pe[:, :, :, None] * g_eq[None, :, None, :]).reshape(
            NB, P, NT * G
        )
        mo = (po[:, :, :, None] * g_eq[None, :, None, :]).reshape(
            NB, P, NT * G
        )

        # delta[p=g*32+h, 3t+j] = s_pts[idx[m,h]] - q[m]
        s_edge = np.where(
            valid.reshape(M_PAD, H, 1), s_pts[idx.reshape(M_PAD, H)], 0.0
        ).astype(np.float32)
        d_e = (s_edge - q[:, None, :]).reshape(NB, NT, G, H, 3)
        delta = np.ascontiguousarray(
            d_e.transpose(0, 2, 3, 1, 4)
        ).reshape(NB, P, NT * 3)

        in_maps.append(
            {
                "tab": tab,
                "idx16": wrap(i16.reshape(NB, NI)),
                "delta": delta,
                "mask_e": me.astype(bf16),
                "mask_o": mo.astype(bf16),
                "weights": w_bf,
                "kp_rep": kp_rep,
            }
        )
    return in_maps


def kernel(q_pts, s_pts, s_feats, neighb_inds, weights, kernel_points):
    global _compiled
    if _compiled is None:
        _compiled = _build_bass()
    nc = _compiled

    from concourse.bass_utils import run_bass_kernel_spmd

    in_maps = _host_prep(
        q_pts, s_pts, s_feats, neighb_inds, weights, kernel_points
    )
    res = run_bass_kernel_spmd(nc, in_maps, core_ids=list(range(N_CORES)))
    out = np.concatenate(
        [r["out"].reshape(M_PAD, C)[:M_CORE] for r in res.results], axis=0
    )
    return out.astype(np.float32)


if __name__ == "__main__":
    rng = np.random.default_rng(0)
    ins = {
        "q_pts": rng.standard_normal((M_TOTAL, 3)).astype(np.float32),
        "s_pts": rng.standard_normal((N_SUP, 3)).astype(np.float32),
        "s_feats": rng.standard_normal((N_SUP, C)).astype(np.float32),
        "neighb_inds": rng.integers(0, N_SUP, (M_TOTAL, H)).astype(np.int32),
        "weights": (rng.standard_normal((K, C, C)) * 0.05).astype(np.float32),
        "kernel_points": rng.standard_normal((K, 3)).astype(np.float32),
    }
    out = kernel(**ins)
    print(out.shape, out.dtype)


# revision 20
# speedup vs baseline: 5.7306x; 2.0496x over previous
"""KPConv Bass/Trainium2 kernel.

out[m,d] = sum_k ( sum_h infl[m,h,k] * s_feats[idx[m,h],:] ) @ W[k]
infl[m,h,k] = relu(1 - |s_pts[idx[m,h]] - q_pts[m] - kp[k]| / SIGMA)

Sharding: query points M=50000 split 8 ways (6250/core, padded to 6272 =
49 blocks x 128 points). Support features / weights replicated per core.

Per-core dataflow, per block of 128 query points (= 32 tiles of 4 points
x 32 neighbors = 128 edges each; edge j of the block sits at gather slot
partition j%128, column j//128, with j = t*128 + g*32 + h):

  1. neighbor features are fetched with ONE dma_gather per block from a
     PAIR-PACKED bf16 table: tab2[p] = [feats[2p] | feats[2p+1]] (512B
     rows, 25001 rows).  Halving the row count fits the whole table in
     dma_gather's int16 index window (25000 < 32768), so a single call
     covers all slots (no split windows / placeholder rows).  dma_gather
     amortizes SWDGE descriptor generation (994ns + 0.34ns/desc per call
     vs ~1us per 128-row indirect DMA in the old kernel), and
     single_packet=False keeps each descriptor its own packet (a
     coalesced packet caps at 64 descriptors -> device error).
  2. the even/odd half-row selection rides the step-A matmul for free:
     bd is built twice, bd_e = infl*mask_e and bd_o = infl*mask_o, where
     the host-shipped per-block masks fold the block-diagonal structure
     AND the row parity: mask_e[p,(t,g)] = (g==p//32)*(idx even).
  3. influence in fp32 on DVE/ACT from host-prepped per-edge delta =
     s_pts[idx]-q (coords ride from the host, 12B/edge, vs 32 tiny 12B
     indirect DMAs per block): (delta-kp)^2, segmented reduce, sqrt,
     relu affine -> infl (bf16).
  4. step A on PE (bf16): per tile t, two accumulating matmuls
     (lhsT=even/odd half of nfp_t [128e,128c], rhs=bd_e/bd_o_t
     [128e, 60]) -> PSUM wfT [128c, m*15+k] fp32 in 4 banks.
  5. step B on PE (bf16): per k, matmul(lhsT=wfT[:, k::15] [c,m],
     rhs=W[k] [c,d]) accumulating over k -> PSUM [128m, 128d] -> SBUF
     -> DRAM.
"""

import sys

sys.path.insert(0, "/opt/trn_rl_repo")

import numpy as np

# ---------------------------------------------------------------- constants
N_CORES = 8
M_TOTAL = 50000
N_SUP = 50000
H = 32
C = 128
K = 15
SIGMA = 2.0

M_CORE = M_TOTAL // N_CORES          # 6250
P = 128                              # partitions / points per block
NB = (M_CORE + P - 1) // P           # 49 blocks
M_PAD = NB * P                       # 6272
G = 4                                # points per step-A matmul tile
NT = P // G                          # 32 tiles per block
NI = P * NT                          # 4096 gathered rows per block

NPAIR = N_SUP // 2                   # 25000 packed feature-pair rows
ZROW = NPAIR                         # all-zero pair row for padded slots

_compiled = None


def _build_bass(nb=NB, compile=True, repeats=1, parts="all"):
    """Build + compile the per-core SPMD Bass program."""
    from contextlib import ExitStack

    import concourse.bacc as bacc
    import concourse.mybir as mybir
    import concourse.tile as tile

    f32 = mybir.dt.float32
    bf16 = mybir.dt.bfloat16
    i16 = mybir.dt.int16
    NB = nb

    nc = bacc.Bacc(
        "TRN2",
        target_bir_lowering=False,
        debug=False,
        enable_asserts=False,
        num_devices=N_CORES,
        num_swdge_queues=4,
    )

    tab_d = nc.dram_tensor("tab", (NPAIR + 1, 2 * C), bf16, kind="ExternalInput")
    idx_d = nc.dram_tensor("idx16", (NB, P, NI // 16), i16, kind="ExternalInput")
    delta_d = nc.dram_tensor("delta", (NB, P, NT * 3), f32, kind="ExternalInput")
    meo_d = nc.dram_tensor("mask_eo", (NB, P, NT * 2 * G), bf16, kind="ExternalInput")
    w_d = nc.dram_tensor("weights", (K, C, C), bf16, kind="ExternalInput")
    kpf_d = nc.dram_tensor("kp_full", (P, NT * K * 3), f32, kind="ExternalInput")
    out_d = nc.dram_tensor("out", (NB, P, C), f32, kind="ExternalOutput")

    sub = mybir.AluOpType.subtract
    mult = mybir.AluOpType.mult

    with tile.TileContext(nc) as tc, ExitStack() as ctx:
        const = ctx.enter_context(tc.tile_pool(name="const", bufs=1))
        io = ctx.enter_context(tc.tile_pool(name="io", bufs=2))
        mid = ctx.enter_context(tc.tile_pool(name="mid", bufs=2))
        psa = ctx.enter_context(tc.tile_pool(name="psa", bufs=1, space="PSUM"))
        psb = ctx.enter_context(tc.tile_pool(name="psb", bufs=2, space="PSUM"))

        # constants: weights as [c, k, d], kernel points replicated per t
        w_sb = const.tile([P, K, C], bf16)
        nc.sync.dma_start(w_sb[:], w_d.ap().rearrange("k c d -> c k d"))
        kpf_sb = const.tile([P, NT * K * 3], f32)
        nc.sync.dma_start(kpf_sb[:], kpf_d.ap())

        NQ = 4                       # SWDGE queues / sub-gathers per block
        NIQ = NI // NQ               # 1024 idxs per sub-gather
        do_gather = parts in ("all", "gather")
        do_compute = parts in ("all", "compute")
        for B in [b for _ in range(repeats) for b in range(NB)]:
            idx = io.tile([P, NI // 16], i16, tag="idx")
            nc.sync.dma_start(idx[:], idx_d.ap()[B])
            dlt = io.tile([P, NT * 3], f32, tag="dlt")
            nc.sync.dma_start(dlt[:], delta_d.ap()[B])
            meo = io.tile([P, NT * 2 * G], bf16, tag="meo")
            nc.sync.dma_start(meo[:], meo_d.ap()[B])

            # gather packed feature pairs: slot j -> [part j%128, col j//128]
            # split across the 4 SWDGE queues so descriptor generation runs
            # on all four Q7 core pairs concurrently (it is the bottleneck);
            # 1024-idx calls keep coalesced packets at the 64-descriptor cap
            nfp = io.tile([P, NT, 2 * C], bf16, tag="nfp")
            if do_gather:
                for qn in range(NQ):
                    nc.gpsimd.dma_gather(
                        out_ap=nfp[:, qn * (NT // NQ) : (qn + 1) * (NT // NQ), :],
                        in_ap=tab_d.ap(),
                        idxs_ap=idx[:, qn * (NIQ // 16) : (qn + 1) * (NIQ // 16)],
                        num_idxs=NIQ,
                        num_idxs_reg=NIQ,
                        elem_size=2 * C,
                        queue_num=qn,
                    )
            else:
                nc.gpsimd.memset(nfp[:], 0.25)
            if not do_compute:
                osb0 = mid.tile([P, C], f32, tag="osb")
                nc.vector.tensor_copy(osb0[:], nfp[:, 0, :C])
                nc.sync.dma_start(out_d.ap()[B], osb0[:])
                continue

            # influence (fp32 math, bf16 out of the relu)
            diff = mid.tile([P, NT * K * 3], f32, tag="diff")
            nc.vector.tensor_tensor(
                diff[:].rearrange("p (t k j) -> p t k j", k=K, j=3),
                dlt[:].rearrange("p (t j) -> p t j", j=3)
                .unsqueeze(2)
                .broadcast_to([P, NT, K, 3]),
                kpf_sb[:].rearrange("p (t k j) -> p t k j", k=K, j=3),
                op=sub,
            )
            sq = mid.tile([P, NT * K * 3], f32, tag="sq")
            nc.scalar.square(sq[:], diff[:])
            d2 = mid.tile([P, NT * K], f32, tag="d2")
            nc.vector.reduce_sum(
                out=d2[:],
                in_=sq[:].rearrange("p (tk j) -> p tk j", j=3),
                axis=mybir.AxisListType.X,
            )
            dd = mid.tile([P, NT * K], f32, tag="dd")
            nc.scalar.sqrt(dd[:], d2[:])
            infl = mid.tile([P, NT * K], bf16, tag="infl")
            nc.scalar.activation(
                infl[:],
                dd[:],
                mybir.ActivationFunctionType.Relu,
                bias=1.0,
                scale=-1.0 / SIGMA,
            )

            # per-parity block-diagonal influence, both parities in one
            # pass: bd_eo[p, t, s, g, k] = infl[p,t,k] * mask_eo[p,t,s,g]
            bd = mid.tile([P, NT * 2 * G * K], bf16, tag="bd")
            nc.vector.tensor_tensor(
                bd[:].rearrange("p (t sg k) -> p t sg k", sg=2 * G, k=K),
                infl[:].rearrange("p (t k) -> p t k", k=K)
                .unsqueeze(2)
                .broadcast_to([P, NT, 2 * G, K]),
                meo[:].rearrange("p (t sg) -> p t sg", sg=2 * G)
                .unsqueeze(3)
                .broadcast_to([P, NT, 2 * G, K]),
                op=mult,
            )
            bdv = bd[:].rearrange("p (t s gk) -> p t s gk", s=2, gk=G * K)

            # step A: per tile, two accumulating matmuls (even/odd halves
            # of the gathered pair rows; the parity masks zero the wrong
            # half's contribution, PSUM accumulation merges them)
            pa = [
                psa.tile([P, 8 * G * K], f32, tag=f"psA{q}", name=f"psA{q}")
                for q in range(4)
            ]
            for t in range(NT):
                dst = pa[t // 8][:, (t % 8) * (G * K) : (t % 8 + 1) * (G * K)]
                nc.tensor.matmul(
                    dst,
                    lhsT=nfp[:, t, :C],
                    rhs=bdv[:, t, 0, :],
                    start=True,
                    stop=False,
                )
                nc.tensor.matmul(
                    dst,
                    lhsT=nfp[:, t, C:],
                    rhs=bdv[:, t, 1, :],
                    start=False,
                    stop=True,
                )
            wfT = mid.tile([P, P * K], bf16, tag="wfT")
            for q in range(4):
                nc.scalar.copy(wfT[:, q * 480 : (q + 1) * 480], pa[q][:])

            # step B: accumulate over k
            outp = psb.tile([P, C], f32, tag="outp")
            wview = wfT[:].rearrange("p (m k) -> p k m", k=K)
            for k in range(K):
                nc.tensor.matmul(
                    outp[:],
                    lhsT=wview[:, k, :],
                    rhs=w_sb[:, k, :],
                    start=(k == 0),
                    stop=(k == K - 1),
                )
            osb = mid.tile([P, C], f32, tag="osb")
            nc.scalar.copy(osb[:], outp[:])
            nc.sync.dma_start(out_d.ap()[B], osb[:])

    if compile:
        nc.compile()
    return nc


def _host_prep(q_pts, s_pts, s_feats, neighb_inds, weights, kernel_points):
    """Shard + lay out inputs for the 8 cores."""
    import ml_dtypes

    bf16 = ml_dtypes.bfloat16

    q_pts = np.asarray(q_pts, np.float32)
    s_pts = np.asarray(s_pts, np.float32)
    s_feats = np.asarray(s_feats, np.float32)
    neighb_inds = np.asarray(neighb_inds, np.int64)
    weights = np.asarray(weights, np.float32)
    kernel_points = np.asarray(kernel_points, np.float32)

    # shared: pair-packed bf16 feature table + all-zero row at the end
    fb = s_feats.astype(bf16)
    tab = np.zeros((NPAIR + 1, 2 * C), bf16)
    tab[:NPAIR, :C] = fb[0::2]
    tab[:NPAIR, C:] = fb[1::2]

    kp_full = np.broadcast_to(
        kernel_points.reshape(1, 1, K * 3), (P, NT, K * 3)
    ).reshape(P, NT * K * 3).copy()
    # g_eq[p, g] = 1 iff partition p belongs to point-group g
    g_eq = (
        (np.arange(G)[None, :] == (np.arange(P)[:, None] // H))
    ).astype(np.float32)
    w_bf = weights.astype(bf16)

    in_maps = []
    for i in range(N_CORES):
        sl = slice(i * M_CORE, (i + 1) * M_CORE)
        q = np.zeros((M_PAD, 3), np.float32)
        q[:M_CORE] = q_pts[sl]
        idx = np.full((M_PAD, H), -1, np.int64)
        idx[:M_CORE] = neighb_inds[sl]

        # gather slot j of block B (= t*128 + g*32 + h, point m =
        # B*128 + t*4 + g) -> ga[B, t, g, h]
        ga = idx.reshape(NB, NT, G, H)
        valid = ga >= 0
        i16 = np.where(valid, ga >> 1, ZROW)

        def wrap(a):
            # [NB, 4096] j-order -> [NB, 128, 4*64] int16: 16-partition
            # wrap within each 1024-slot sub-gather chunk, replicated x8
            w = a.reshape(NB, 4, 64, 16).transpose(0, 3, 1, 2)  # [B,16,4,64]
            w = w.reshape(NB, 16, NI // 16)
            return np.ascontiguousarray(
                np.tile(w, (1, 8, 1)), dtype=np.int16
            )

        # parity [B, t, g, h] -> [B, p=(g,h), t]
        pe = (valid & (ga % 2 == 0)).transpose(0, 2, 3, 1)  # [B,G,H,T]
        po = (valid & (ga % 2 == 1)).transpose(0, 2, 3, 1)
        pe = pe.reshape(NB, P, NT).astype(np.float32)
        po = po.reshape(NB, P, NT).astype(np.float32)
        # mask_eo[B, p, (t, s, g)] = parity_s[B, p, t] * g_eq[p, g]
        ps = np.stack([pe, po], axis=3)  # [B, P, T, 2]
        meo = (ps[:, :, :, :, None] * g_eq[None, :, None, None, :]).reshape(
            NB, P, NT * 2 * G
        )

        # delta[p=g*32+h, 3t+j] = s_pts[idx[m,h]] - q[m]
        s_edge = np.where(
            valid.reshape(M_PAD, H, 1), s_pts[idx.reshape(M_PAD, H)], 0.0
        ).astype(np.float32)
        d_e = (s_edge - q[:, None, :]).reshape(NB, NT, G, H, 3)
        delta = np.ascontiguousarray(
            d_e.transpose(0, 2, 3, 1, 4)
        ).reshape(NB, P, NT * 3)

        in_maps.append(
            {
                "tab": tab,
                "idx16": wrap(i16.reshape(NB, NI)),
                "delta": delta,
                "mask_eo": meo.astype(bf16),
                "weights": w_bf,
                "kp_full": kp_full,
            }
        )
    return in_maps


def kernel(q_pts, s_pts, s_feats, neighb_inds, weights, kernel_points):
    global _compiled
    if _compiled is None:
        _compiled = _build_bass()
    nc = _compiled

    from concourse.bass_utils import run_bass_kernel_spmd

    in_maps = _host_prep(
        q_pts, s_pts, s_feats, neighb_inds, weights, kernel_points
    )
    res = run_bass_kernel_spmd(nc, in_maps, core_ids=list(range(N_CORES)))
    out = np.concatenate(
        [r["out"].reshape(M_PAD, C)[:M_CORE] for r in res.results], axis=0
    )
    return out.astype(np.float32)


if __name__ == "__main__":
    rng = np.random.default_rng(0)
    ins = {
        "q_pts": rng.standard_normal((M_TOTAL, 3)).astype(np.float32),
        "s_pts": rng.standard_normal((N_SUP, 3)).astype(np.float32),
        "s_feats": rng.standard_normal((N_SUP, C)).astype(np.float32),
        "neighb_inds": rng.integers(0, N_SUP, (M_TOTAL, H)).astype(np.int32),
        "weights": (rng.standard_normal((K, C, C)) * 0.05).astype(np.float32),
        "kernel_points": rng.standard_normal((K, 3)).astype(np.float32),
    }
    out = kernel(**ins)
    print(out.shape, out.dtype)
